# revision 31
# baseline (speedup 1.0000x reference)
"""Causal self-attention Trainium2 kernel (8-core SPMD), bf16 edition.

Reference: y = softmax(mask(q k^T / sqrt(dh))) v -> proj, with
x [B=4, T=2048, C=1024], H=16 heads, dh=64.

Sharding: core i handles batch b = i//2 and head-group g = i%2 (8 heads).
Each core computes a partial y (its heads' contribution to the output
projection); the host sums the two partials per batch and adds proj_b.

Engine plan (attention is ACT-exp-paced at ~(1024+352)/1.2 ns per
2-chunk pair):
  PE:    projections, S^T = K_chunk @ Q^T (lh0/lh1 run concurrently on
         disjoint row-groups since dh=64), PV accumulation, plus QK
         projection tiles for later head-pairs injected as *filler*
         between attention steps so PE never idles and HAM stays warm.
  ACT:   exp only (plus a table-preload dummy at t~0).
  DVE:   bias adds, tril masks on diag chunks, PSUM evict/stash.
  GPSIMD: O *= 1/rowsum (bf16, SBUF-only) after the per-hp rowsum dance.
  SYNC/GPSIMD queues: DMAs (bf16 inputs halve traffic; fp32 y out).
"""

import numpy as np
import ml_dtypes

C = 1024
HLOC = 8
DH = 64
QB = 512  # query block (PSUM bank width in fp32)
KC = 128  # key chunk (partition dim)

_cache = {}


def _build(T, mm_dt="bf16", dbg=False):
    import concourse.bass as bass
    import concourse.tile as tile
    from concourse import bacc, mybir

    f32 = mybir.dt.float32
    bf16 = mybir.dt.bfloat16
    nqb = T // QB
    ctiles = C // 128
    ttiles = T // 128

    nc = bacc.Bacc("TRN2", target_bir_lowering=False, debug=False)

    xT = nc.dram_tensor("xT", [C, T], bf16, kind="ExternalInput")
    wqkT = nc.dram_tensor("wqkT", [C // 128, 128, C // 128, 128], bf16, kind="ExternalInput")
    wvT = nc.dram_tensor("wvT", [C, 512], bf16, kind="ExternalInput")
    woT = nc.dram_tensor("woT", [512, C], bf16, kind="ExternalInput")
    qkb = nc.dram_tensor("qkb", [C], f32, kind="ExternalInput")
    vb = nc.dram_tensor("vb", [512], f32, kind="ExternalInput")
    tril = nc.dram_tensor("tril", [128, 128], bf16, kind="ExternalInput")
    ident = nc.dram_tensor("ident", [128, 128], bf16, kind="ExternalInput")
    ntriu = nc.dram_tensor("ntriu", [128, 128], bf16, kind="ExternalInput")
    y = nc.dram_tensor("y", [T, C], f32, kind="ExternalOutput")

    Exp = mybir.ActivationFunctionType.Exp

    with tile.TileContext(nc) as tc:
        with (
            tc.tile_pool(name="persist", bufs=1) as persist,
            tc.tile_pool(name="consts", bufs=1) as consts,
        ):
            qkT_sb = persist.tile([128, ctiles, T], bf16)
            V_sb = persist.tile([128, T // 128, HLOC * 65], bf16)
            tril_sb = consts.tile([128, 128], bf16)
            qkb_sb = consts.tile([128, ctiles], f32)
            vb_sb = consts.tile([128, 512], f32)
            scratch = consts.tile([128, 512], bf16)

            nc.gpsimd.dma_start(tril_sb[:], tril[:])
            ident_sb = consts.tile([128, 128], bf16)
            ntriu_sb = consts.tile([128, 128], bf16)
            nc.gpsimd.dma_start(ident_sb[:], ident[:])
            nc.gpsimd.dma_start(ntriu_sb[:], ntriu[:])
            nc.gpsimd.dma_start(qkb_sb[:], qkb.rearrange("(r p) -> p r", p=128))
            vb_ap = vb[:]
            nc.gpsimd.dma_start(
                vb_sb[:],
                bass.AP(
                    tensor=vb_ap.tensor, offset=vb_ap.offset, ap=[[0, 128], [1, 512]]
                ),
            )
            # preload the exp table-set (~2.7us) long before the first
            # real exp; scratch also serves as warm-keeper matmul operand.
            nc.scalar.activation(scratch[:], vb_sb[:], Exp)

            # ones columns of V (col 64 of each head's 65-wide slot).
            v_grp = V_sb.rearrange("p t (h c) -> p t h c", c=65)
            nc.scalar.activation(
                v_grp[:, :, :, 64:65],
                tril_sb[:, 0 : (T // 128) * HLOC].rearrange(
                    "p (a b c) -> p a b c", a=T // 128, b=HLOC, c=1
                ),
                mybir.ActivationFunctionType.Copy,
                bias=1.0,
                scale=0.0,
            )

            # -------- input DMAs (bf16) --------
            with (
                tc.tile_pool(name="xw", bufs=1) as xw,
                tc.tile_pool(name="wqks", bufs=2) as wqks,
                tc.tile_pool(name="ot", bufs=1) as ot,
            ):
                xT_sb = xw.tile([128, ctiles, T], bf16)
                wvT_sb = xw.tile([128, ctiles, 512], bf16)
                OT_sb = ot.tile([128, 4, T], bf16)
                woT_sb = ot.tile([128, 4, C], bf16)

                # wqk tiles for rt0 (Q of hp0) and rt4 (K of hp0) up front
                wqk_tiles = {}

                def load_wqk(rt, eng=nc.sync):
                    t = wqks.tile([128, ctiles, 128], bf16, tag="wqk", name=f"wqk{rt}")
                    eng.dma_start(t[:], wqkT[rt])
                    wqk_tiles[rt] = t

                load_wqk(0)
                xT_r = xT.rearrange("(c p) t -> p c t", p=128)
                for c in range(ctiles):
                    eng = (nc.sync, nc.gpsimd, nc.scalar)[c % 3]
                    eng.dma_start(xT_sb[:, c, :], xT_r[:, c, :])
                load_wqk(4)
                nc.gpsimd.dma_start(wvT_sb[:], wvT.rearrange("(c p) v -> p c v", p=128))
                nc.scalar.dma_start(woT_sb[:], woT.rearrange("(c p) o -> p c o", p=128))

                with (
                    tc.tile_pool(name="pexp", bufs=6) as pexp,
                    tc.tile_pool(name="rsbp", bufs=4) as rsbp,
                    tc.tile_pool(name="yp", bufs=4) as yp,
                    tc.tile_pool(name="psS", bufs=2, space="PSUM") as psS,
                    tc.tile_pool(name="psO", bufs=2, space="PSUM") as psO,
                    tc.tile_pool(name="pj", bufs=2, space="PSUM") as pj,
                ):
                    # tiny PE warmup while first xT tile lands (depends only
                    # on the small tril DMA, first in the gpsimd queue)
                    for w in range(6):
                        wp = pj.tile([128, 512], f32, tag="pj", name=f"warm{w}")
                        nc.tensor.matmul(
                            wp[:, 0:128], tril_sb[:], tril_sb[:], start=True, stop=True
                        )

                    def qk_block(rt, nt):
                        """one [128,512] block of the QK projection"""
                        ps = pj.tile([128, 512], f32, tag="pj", name=f"qk{rt}_{nt}")
                        wt = wqk_tiles[rt]
                        for c in range(ctiles):
                            nc.tensor.matmul(
                                ps[:],
                                wt[:, c, :],
                                xT_sb[:, c, nt * 512 : (nt + 1) * 512],
                                start=(c == 0),
                                stop=(c == ctiles - 1),
                            )
                        nc.vector.tensor_scalar_add(
                            qkT_sb[:, rt, nt * 512 : (nt + 1) * 512],
                            ps[:],
                            qkb_sb[:, rt : rt + 1],
                        )

                    def v_block(tt):
                        ps = pj.tile([128, 512], f32, tag="pj", name=f"v{tt}")
                        for c in range(ctiles):
                            nc.tensor.matmul(
                                ps[:],
                                xT_sb[:, c, tt * 128 : (tt + 1) * 128],
                                wvT_sb[:, c, :],
                                start=(c == 0),
                                stop=(c == ctiles - 1),
                            )
                        nc.vector.tensor_add(
                            v_grp[:, tt, :, 0:64],
                            ps.rearrange("p (h c) -> p h c", c=64),
                            vb_sb.rearrange("p (h c) -> p h c", c=64),
                        )

                    # ---- pre-attention: QK for hp0, all of V ----
                    for nt in range(nqb):
                        qk_block(0, nt)
                        qk_block(4, nt)
                    for tt in range(ttiles):
                        v_block(tt)

                    # ---- filler machine: QK blocks for hp1..3 first, then
                    # output-projection blocks as their OT columns unlock,
                    # then warm-keeper matmuls. One matmul per emit so the
                    # granularity matches per-step PE slack; rt weight tiles
                    # are prefetched one rt ahead.
                    fill_rts = [1, 5, 2, 6, 3, 7]
                    fill_blocks = [(ri, nt) for ri in range(6) for nt in range(nqb)]
                    fst = {
                        "blk": 0, "c": 0, "ps": None, "junk": 0, "loaded": 1,
                        "yq": [], "yi": 0, "yc": 0, "yps": None, "drain": False,
                    }
                    load_wqk(fill_rts[0], eng=nc.sync)

                    def emit_y_mm():
                        tt, nt = fst["yq"][fst["yi"]]
                        if fst["yps"] is None:
                            fst["yps"] = pj.tile(
                                [128, 512], f32, tag="pj", name=f"y{tt}_{nt}"
                            )
                        c4 = fst["yc"]
                        nc.tensor.matmul(
                            fst["yps"][:],
                            OT_sb[:, c4, tt * 128 : (tt + 1) * 128],
                            woT_sb[:, c4, nt * 512 : (nt + 1) * 512],
                            start=(c4 == 0),
                            stop=(c4 == 3),
                        )
                        fst["yc"] += 1
                        if fst["yc"] == 4:
                            yt = yp.tile([128, 512], f32, tag="yt")
                            if fst["drain"]:
                                # ACT is idle once attention exps are done;
                                # keep DVE clear for epilogue evicts
                                nc.scalar.activation(
                                    yt[:], fst["yps"][:],
                                    mybir.ActivationFunctionType.Copy,
                                )
                            else:
                                nc.vector.tensor_copy(yt[:], fst["yps"][:])
                            eng = (nc.sync, nc.gpsimd, nc.scalar)[(tt * 2 + nt) % 3]
                            eng.dma_start(
                                y[tt * 128 : (tt + 1) * 128, nt * 512 : (nt + 1) * 512],
                                yt[:],
                            )
                            fst["yps"] = None
                            fst["yc"] = 0
                            fst["yi"] += 1

                    def emit_filler(n, junk_ok=True, allow_y=True):
                        for _ in range(n):
                            if fst["blk"] < len(fill_blocks):
                                ri, nt = fill_blocks[fst["blk"]]
                                rt = fill_rts[ri]
                                if fst["c"] == 0 and fst["ps"] is None:
                                    if nt == 0 and fst["loaded"] == ri + 1 and ri + 1 < 6:
                                        # starting a new rt: prefetch next one
                                        load_wqk(fill_rts[ri + 1], eng=nc.sync)
                                        fst["loaded"] = ri + 2
                                    fst["ps"] = pj.tile(
                                        [128, 512], f32, tag="pj", name=f"f{rt}_{nt}"
                                    )
                                c = fst["c"]
                                nc.tensor.matmul(
                                    fst["ps"][:],
                                    wqk_tiles[rt][:, c, :],
                                    xT_sb[:, c, nt * 512 : (nt + 1) * 512],
                                    start=(c == 0),
                                    stop=(c == ctiles - 1),
                                )
                                fst["c"] += 1
                                if fst["c"] == ctiles:
                                    nc.vector.tensor_scalar_add(
                                        qkT_sb[:, rt, nt * 512 : (nt + 1) * 512],
                                        fst["ps"][:],
                                        qkb_sb[:, rt : rt + 1],
                                    )
                                    fst["ps"] = None
                                    fst["c"] = 0
                                    fst["blk"] += 1
                            elif fst["yi"] < len(fst["yq"]) and (
                                allow_y or fst["yc"] > 0
                            ):
                                emit_y_mm()
                            elif junk_ok and fst["junk"] < 24:
                                # warm-keeper matmul (result unused)
                                fst["junk"] += 1
                                wp = pj.tile(
                                    [128, 512], f32, tag="pj",
                                    name=f"junk{fst['junk']}",
                                )
                                nc.tensor.matmul(
                                    wp[:], tril_sb[:], scratch[:], start=True, stop=True
                                )
                            else:
                                return

                    # ---- attention ----
                    for hp in range(4):
                        for qb in range(nqb):
                            po = [
                                psO.tile([65, 512], f32, tag="po", name=f"po{hp}_{qb}_{i}")
                                for i in range(2)
                            ]
                            nkc = (qb + 1) * (QB // KC)
                            stage = []  # (ps, lh, kcp) pending exp
                            pvq = []  # (pt, lh, kcp) pending PV, one extra
                            # step behind so the first PV's wait on the po
                            # evict never stalls the in-order PE queue
                            for kcp in range(nkc // 2 + 2):
                                if kcp < nkc // 2:
                                    # filler BEFORE the S pairs; the four S
                                    # matmuls stay contiguous so the lh0/lh1
                                    # halves (row groups 0-1 vs 2-3, dh=64)
                                    # overlap in the PE array
                                    emit_filler(4)
                                    for lh in range(2):
                                        b0 = 64 * lh
                                        ps = psS.tile([128, 1024], f32)
                                        for j in range(2):
                                            kc = 2 * kcp + j
                                            o = kc * 128 - qb * 512
                                            lo = max(o, 0)
                                            pe_mask = o >= 0 and qb == 0
                                            if pe_mask:
                                                # qb0: seed -1e9 above the
                                                # diagonal on PE (a DVE mask
                                                # here would queue behind the
                                                # previous qb's epilogue)
                                                nc.tensor.matmul(
                                                    ps[:, j * 512 + lo : j * 512 + lo + 128],
                                                    ident_sb[:],
                                                    ntriu_sb[:],
                                                    start=True,
                                                    stop=False,
                                                )
                                            nc.tensor.matmul(
                                                ps[:, j * 512 + lo : (j + 1) * 512],
                                                qkT_sb[
                                                    b0 : b0 + 64,
                                                    4 + hp,
                                                    kc * 128 : (kc + 1) * 128,
                                                ],
                                                qkT_sb[
                                                    b0 : b0 + 64,
                                                    hp,
                                                    qb * 512 + lo : (qb + 1) * 512,
                                                ],
                                                start=(not pe_mask),
                                                stop=True,
                                            )
                                        stage.append((ps, lh, kcp))
                                if kcp > 0 and stage:
                                    ready, stage = stage[:2], stage[2:]
                                    for ps, lh, pkcp in ready:
                                        pt = pexp.tile([128, 1024], bf16)
                                        nc.scalar.activation(pt[:], ps[:], Exp)
                                        for j in range(2):
                                            o = (2 * pkcp + j) * 128 - qb * 512
                                            if o >= 0 and qb > 0:
                                                # diag mask on DVE (cheap; no
                                                # epilogue collision off qb0)
                                                nc.vector.tensor_mul(
                                                    pt[:, j * 512 + o : j * 512 + o + 128],
                                                    pt[:, j * 512 + o : j * 512 + o + 128],
                                                    tril_sb[:],
                                                )
                                        pvq.append((pt, lh, pkcp))
                                if kcp > 1 and pvq:
                                    pvready, pvq = pvq[:2], pvq[2:]
                                    for pt, lh, pkcp in pvready:
                                        for j in range(2):
                                            kc = 2 * pkcp + j
                                            o = kc * 128 - qb * 512
                                            lo = max(o, 0)
                                            nc.tensor.matmul(
                                                po[lh][:, lo:512],
                                                V_sb[
                                                    :,
                                                    kc,
                                                    (2 * hp + lh) * 65 : (2 * hp + lh) * 65
                                                    + 65,
                                                ],
                                                pt[:, j * 512 + lo : (j + 1) * 512],
                                                start=(kc == 0),
                                                stop=(kc == nkc - 1),
                                            )

                            # per-(hp,qb) epilogue: reciprocal of rowsums
                            # straight from PSUM, evict unnormalized O^T,
                            # broadcast 1/rowsum, normalize in place (GPSIMD)
                            q_sl = slice(qb * 512, (qb + 1) * 512)
                            idx = hp * nqb + qb
                            rs = rsbp.tile([65, 512], f32, tag="rs")
                            for lh in range(2):
                                nc.vector.tensor_copy(
                                    rs[64 * lh : 64 * lh + 1, :], po[lh][64:65, :]
                                )
                            # broadcast RAW rowsums right away (DMA latency
                            # hides under the evict copies below), then a fast
                            # approx reciprocal on the 128-lane tile
                            rsb = rsbp.tile([128, 512], f32, tag="rsb")
                            for lh in range(2):
                                row = rs[64 * lh : 64 * lh + 1, :]
                                src = bass.AP(
                                    tensor=row.tensor,
                                    offset=row.offset,
                                    ap=[list(row.ap[0]), [0, 64], [1, 512]],
                                )
                                eng = (nc.sync, nc.gpsimd)[idx % 2]
                                eng.dma_start(rsb[64 * lh : 64 * lh + 64, :], src)
                            for lh in range(2):
                                nc.vector.tensor_copy(
                                    OT_sb[64 * lh : 64 * lh + 64, hp, q_sl],
                                    po[lh][0:64, :],
                                )
                            nc.vector.reciprocal_approx_fast(rsb[:, :], rsb[:, :])
                            nc.gpsimd.tensor_mul(
                                OT_sb[:, hp, q_sl],
                                OT_sb[:, hp, q_sl],
                                rsb[:, :],
                            )
                            if hp == 3:
                                # this qb's OT columns are now final for all
                                # head pairs: unlock its output-proj blocks
                                for tt in range(qb * 4, qb * 4 + 4):
                                    fst["yq"].append((tt, 0))
                                    fst["yq"].append((tt, 1))

                    # drain remaining output-projection work
                    fst["drain"] = True
                    while fst["yi"] < len(fst["yq"]) or fst["blk"] < len(fill_blocks):
                        emit_filler(4, junk_ok=False)

    nc.compile()
    return nc


def get_nc(T=2048, mm_dt="bf16", dbg=False):
    key = (T, mm_dt, dbg)
    if key not in _cache:
        _cache[key] = _build(T, mm_dt, dbg)
    return _cache[key]


def make_in_maps(x, qkv_w, qkv_b, proj_w, proj_b):
    B, T, _ = x.shape
    f = np.float32
    bf = ml_dtypes.bfloat16
    # S^T blocks are [key, query]: keep k <= q  ->  upper triangle
    tril = np.triu(np.ones((128, 128), f)).astype(bf)
    ident = np.eye(128, dtype=f).astype(bf)
    # -1e9 on the masked (k > q) strict-lower part of the S^T block
    ntriu = (-1e9 * (1.0 - np.triu(np.ones((128, 128), f)))).astype(bf)
    in_maps = []
    for i in range(B * 2):
        b, g = i // 2, i % 2
        sl = slice(g * 512, (g + 1) * 512)
        wq = qkv_w[0 * C : 1 * C][sl] * (1.0 / 8.0)
        wk = qkv_w[1 * C : 2 * C][sl]
        wv = qkv_w[2 * C : 3 * C][sl]
        in_maps.append(
            {
                "xT": np.ascontiguousarray(x[b].T).astype(bf),
                "wqkT": np.ascontiguousarray(
                    np.stack(
                        [
                            np.concatenate([wq, wk], 0)
                            .T[:, rt * 128 : (rt + 1) * 128]
                            .reshape(C // 128, 128, 128)
                            .transpose(1, 0, 2)
                            for rt in range(C // 128)
                        ]
                    )
                ).astype(bf),
                "wvT": np.ascontiguousarray(wv.T).astype(bf),
                "woT": np.ascontiguousarray(proj_w[:, sl].T).astype(bf),
                "qkb": np.concatenate(
                    [qkv_b[0 * C : 1 * C][sl] * (1.0 / 8.0), qkv_b[1 * C : 2 * C][sl]]
                ).astype(f),
                "vb": qkv_b[2 * C : 3 * C][sl].astype(f),
                "tril": tril,
                "ident": ident,
                "ntriu": ntriu,
            }
        )
    return in_maps


def kernel(x, qkv_w, qkv_b, proj_w, proj_b, mm_dt="bf16", trace=False, tmpdir=None):
    from concourse.bass_utils import run_bass_kernel_spmd

    x = np.asarray(x, np.float32)
    qkv_w = np.asarray(qkv_w, np.float32)
    qkv_b = np.asarray(qkv_b, np.float32)
    proj_w = np.asarray(proj_w, np.float32)
    proj_b = np.asarray(proj_b, np.float32)

    B, T, _ = x.shape
    nc = get_nc(T, mm_dt)
    in_maps = make_in_maps(x, qkv_w, qkv_b, proj_w, proj_b)
    res = run_bass_kernel_spmd(
        nc, in_maps, list(range(len(in_maps))), trace=trace, tmpdir=tmpdir
    )
    out = np.empty((B, T, C), np.float32)
    for b in range(B):
        out[b] = res.results[2 * b]["y"] + res.results[2 * b + 1]["y"] + proj_b
    kernel.last_result = res
    return out


# revision 33
# speedup vs baseline: 1.0675x; 1.0675x over previous
"""Causal self-attention Trainium2 kernel (8-core SPMD), bf16 edition.

Reference: y = softmax(mask(q k^T / sqrt(dh))) v -> proj, with
x [B=4, T=2048, C=1024], H=16 heads, dh=64.

Sharding: core i handles batch b = i//2 and head-group g = i%2 (8 heads).
Each core computes a partial y (its heads' contribution to the output
projection); the host sums the two partials per batch and adds proj_b.

Engine plan (attention is ACT-exp-paced at ~(1024+352)/1.2 ns per
2-chunk pair):
  PE:    projections, S^T = K_chunk @ Q^T (lh0/lh1 run concurrently on
         disjoint row-groups since dh=64), PV accumulation, plus QK
         projection tiles for later head-pairs injected as *filler*
         between attention steps so PE never idles and HAM stays warm.
  ACT:   exp only (plus a table-preload dummy at t~0).
  DVE:   bias adds, tril masks on diag chunks, PSUM evict/stash.
  GPSIMD: O *= 1/rowsum (bf16, SBUF-only) after the per-hp rowsum dance.
  SYNC/GPSIMD queues: DMAs (bf16 inputs halve traffic; fp32 y out).
"""

import numpy as np
import ml_dtypes

C = 1024
HLOC = 8
DH = 64
QB = 512  # query block (PSUM bank width in fp32)
KC = 128  # key chunk (partition dim)

_cache = {}


def _build(T, mm_dt="bf16", dbg=False):
    import concourse.bass as bass
    import concourse.tile as tile
    from concourse import bacc, mybir

    f32 = mybir.dt.float32
    bf16 = mybir.dt.bfloat16
    nqb = T // QB
    ctiles = C // 128
    ttiles = T // 128

    nc = bacc.Bacc("TRN2", target_bir_lowering=False, debug=False)

    xT = nc.dram_tensor("xT", [C, T], bf16, kind="ExternalInput")
    wqkT = nc.dram_tensor("wqkT", [C // 128, 128, C // 128, 128], bf16, kind="ExternalInput")
    wvT = nc.dram_tensor("wvT", [C, 512], bf16, kind="ExternalInput")
    woT = nc.dram_tensor("woT", [512, C], bf16, kind="ExternalInput")
    qkb = nc.dram_tensor("qkb", [C], f32, kind="ExternalInput")
    vb = nc.dram_tensor("vb", [512], f32, kind="ExternalInput")
    tril = nc.dram_tensor("tril", [128, 128], bf16, kind="ExternalInput")
    ident = nc.dram_tensor("ident", [128, 128], bf16, kind="ExternalInput")
    ntriu = nc.dram_tensor("ntriu", [128, 128], bf16, kind="ExternalInput")
    y = nc.dram_tensor("y", [T, C], f32, kind="ExternalOutput")

    Exp = mybir.ActivationFunctionType.Exp

    with tile.TileContext(nc) as tc:
        with (
            tc.tile_pool(name="persist", bufs=1) as persist,
            tc.tile_pool(name="consts", bufs=1) as consts,
        ):
            qkT_sb = persist.tile([128, ctiles, T], bf16)
            V_sb = persist.tile([128, T // 128, HLOC * 65], bf16)
            tril_sb = consts.tile([128, 128], bf16)
            qkb_sb = consts.tile([128, ctiles], f32)
            vb_sb = consts.tile([128, 512], f32)
            scratch = consts.tile([128, 512], bf16)

            nc.gpsimd.dma_start(tril_sb[:], tril[:])
            ident_sb = consts.tile([128, 128], bf16)
            ntriu_sb = consts.tile([128, 128], bf16)
            nc.gpsimd.dma_start(ident_sb[:], ident[:])
            nc.gpsimd.dma_start(ntriu_sb[:], ntriu[:])
            nc.gpsimd.dma_start(qkb_sb[:], qkb.rearrange("(r p) -> p r", p=128))
            vb_ap = vb[:]
            nc.gpsimd.dma_start(
                vb_sb[:],
                bass.AP(
                    tensor=vb_ap.tensor, offset=vb_ap.offset, ap=[[0, 128], [1, 512]]
                ),
            )
            # preload the exp table-set (~2.7us) long before the first
            # real exp; scratch also serves as warm-keeper matmul operand.
            nc.scalar.activation(scratch[:], vb_sb[:], Exp)

            # ones columns of V (col 64 of each head's 65-wide slot).
            v_grp = V_sb.rearrange("p t (h c) -> p t h c", c=65)
            nc.scalar.activation(
                v_grp[:, :, :, 64:65],
                tril_sb[:, 0 : (T // 128) * HLOC].rearrange(
                    "p (a b c) -> p a b c", a=T // 128, b=HLOC, c=1
                ),
                mybir.ActivationFunctionType.Copy,
                bias=1.0,
                scale=0.0,
            )

            # -------- input DMAs (bf16) --------
            with (
                tc.tile_pool(name="xw", bufs=1) as xw,
                tc.tile_pool(name="wqks", bufs=2) as wqks,
                tc.tile_pool(name="ot", bufs=1) as ot,
            ):
                xT_sb = xw.tile([128, ctiles, T], bf16)
                wvT_sb = xw.tile([128, ctiles, 512], bf16)
                OT_sb = ot.tile([128, 4, T], bf16)
                woT_sb = ot.tile([128, 4, C], bf16)

                # wqk tiles for rt0 (Q of hp0) and rt4 (K of hp0) up front
                wqk_tiles = {}

                def load_wqk(rt, eng=nc.sync):
                    t = wqks.tile([128, ctiles, 128], bf16, tag="wqk", name=f"wqk{rt}")
                    eng.dma_start(t[:], wqkT[rt])
                    wqk_tiles[rt] = t

                load_wqk(0)
                xT_r = xT.rearrange("(c p) t -> p c t", p=128)
                for c in range(ctiles):
                    eng = (nc.sync, nc.gpsimd, nc.scalar)[c % 3]
                    eng.dma_start(xT_sb[:, c, :], xT_r[:, c, :])
                load_wqk(4)
                nc.gpsimd.dma_start(wvT_sb[:], wvT.rearrange("(c p) v -> p c v", p=128))
                nc.scalar.dma_start(woT_sb[:], woT.rearrange("(c p) o -> p c o", p=128))

                with (
                    tc.tile_pool(name="pexp", bufs=6) as pexp,
                    tc.tile_pool(name="rsbp", bufs=4) as rsbp,
                    tc.tile_pool(name="yp", bufs=4) as yp,
                    tc.tile_pool(name="psS", bufs=2, space="PSUM") as psS,
                    tc.tile_pool(name="psO", bufs=2, space="PSUM") as psO,
                    tc.tile_pool(name="pj", bufs=2, space="PSUM") as pj,
                ):
                    # tiny PE warmup while first xT tile lands (depends only
                    # on the small tril DMA, first in the gpsimd queue)
                    for w in range(6):
                        wp = pj.tile([128, 512], f32, tag="pj", name=f"warm{w}")
                        nc.tensor.matmul(
                            wp[:, 0:128], tril_sb[:], tril_sb[:], start=True, stop=True
                        )

                    def qk_block(rt, nt):
                        """one [128,512] block of the QK projection"""
                        ps = pj.tile([128, 512], f32, tag="pj", name=f"qk{rt}_{nt}")
                        wt = wqk_tiles[rt]
                        for c in range(ctiles):
                            nc.tensor.matmul(
                                ps[:],
                                wt[:, c, :],
                                xT_sb[:, c, nt * 512 : (nt + 1) * 512],
                                start=(c == 0),
                                stop=(c == ctiles - 1),
                            )
                        nc.vector.tensor_scalar_add(
                            qkT_sb[:, rt, nt * 512 : (nt + 1) * 512],
                            ps[:],
                            qkb_sb[:, rt : rt + 1],
                        )

                    def v_block(tt):
                        ps = pj.tile([128, 512], f32, tag="pj", name=f"v{tt}")
                        for c in range(ctiles):
                            nc.tensor.matmul(
                                ps[:],
                                xT_sb[:, c, tt * 128 : (tt + 1) * 128],
                                wvT_sb[:, c, :],
                                start=(c == 0),
                                stop=(c == ctiles - 1),
                            )
                        nc.vector.tensor_add(
                            v_grp[:, tt, :, 0:64],
                            ps.rearrange("p (h c) -> p h c", c=64),
                            vb_sb.rearrange("p (h c) -> p h c", c=64),
                        )

                    # ---- pre-attention: QK for hp0, all of V ----
                    for nt in range(nqb):
                        qk_block(0, nt)
                        qk_block(4, nt)
                    for tt in range(ttiles):
                        v_block(tt)

                    # ---- filler machine: QK blocks for hp1..3 first, then
                    # output-projection blocks as their OT columns unlock,
                    # then warm-keeper matmuls. One matmul per emit so the
                    # granularity matches per-step PE slack; rt weight tiles
                    # are prefetched one rt ahead.
                    fill_rts = [1, 5, 2, 6, 3, 7]
                    fill_blocks = [(ri, nt) for ri in range(6) for nt in range(nqb)]
                    fst = {
                        "blk": 0, "c": 0, "ps": None, "junk": 0, "loaded": 1,
                        "yq": [], "yi": 0, "yc": 0, "yps": None, "drain": False,
                    }
                    load_wqk(fill_rts[0], eng=nc.sync)

                    def emit_y_mm():
                        tt, nt = fst["yq"][fst["yi"]]
                        if fst["yps"] is None:
                            fst["yps"] = pj.tile(
                                [128, 512], f32, tag="pj", name=f"y{tt}_{nt}"
                            )
                        c4 = fst["yc"]
                        nc.tensor.matmul(
                            fst["yps"][:],
                            OT_sb[:, c4, tt * 128 : (tt + 1) * 128],
                            woT_sb[:, c4, nt * 512 : (nt + 1) * 512],
                            start=(c4 == 0),
                            stop=(c4 == 3),
                        )
                        fst["yc"] += 1
                        if fst["yc"] == 4:
                            yt = yp.tile([128, 512], f32, tag="yt")
                            if fst["drain"]:
                                # ACT is idle once attention exps are done;
                                # keep DVE clear for epilogue evicts
                                nc.scalar.activation(
                                    yt[:], fst["yps"][:],
                                    mybir.ActivationFunctionType.Copy,
                                )
                            else:
                                nc.vector.tensor_copy(yt[:], fst["yps"][:])
                            eng = (nc.sync, nc.gpsimd, nc.scalar)[(tt * 2 + nt) % 3]
                            eng.dma_start(
                                y[tt * 128 : (tt + 1) * 128, nt * 512 : (nt + 1) * 512],
                                yt[:],
                            )
                            fst["yps"] = None
                            fst["yc"] = 0
                            fst["yi"] += 1

                    def emit_filler(n, junk_ok=True, allow_y=True):
                        for _ in range(n):
                            if fst["blk"] < len(fill_blocks):
                                ri, nt = fill_blocks[fst["blk"]]
                                rt = fill_rts[ri]
                                if fst["c"] == 0 and fst["ps"] is None:
                                    if nt == 0 and fst["loaded"] == ri + 1 and ri + 1 < 6:
                                        # starting a new rt: prefetch next one
                                        load_wqk(fill_rts[ri + 1], eng=nc.sync)
                                        fst["loaded"] = ri + 2
                                    fst["ps"] = pj.tile(
                                        [128, 512], f32, tag="pj", name=f"f{rt}_{nt}"
                                    )
                                c = fst["c"]
                                nc.tensor.matmul(
                                    fst["ps"][:],
                                    wqk_tiles[rt][:, c, :],
                                    xT_sb[:, c, nt * 512 : (nt + 1) * 512],
                                    start=(c == 0),
                                    stop=(c == ctiles - 1),
                                )
                                fst["c"] += 1
                                if fst["c"] == ctiles:
                                    nc.vector.tensor_scalar_add(
                                        qkT_sb[:, rt, nt * 512 : (nt + 1) * 512],
                                        fst["ps"][:],
                                        qkb_sb[:, rt : rt + 1],
                                    )
                                    fst["ps"] = None
                                    fst["c"] = 0
                                    fst["blk"] += 1
                            elif fst["yi"] < len(fst["yq"]) and (
                                allow_y or fst["yc"] > 0
                            ):
                                emit_y_mm()
                            elif junk_ok and fst["junk"] < 24:
                                # warm-keeper matmul (result unused)
                                fst["junk"] += 1
                                wp = pj.tile(
                                    [128, 512], f32, tag="pj",
                                    name=f"junk{fst['junk']}",
                                )
                                nc.tensor.matmul(
                                    wp[:], tril_sb[:], scratch[:], start=True, stop=True
                                )
                            else:
                                return

                    # ---- attention ----
                    for hp in range(4):
                        for qb in range(nqb):
                            po = [
                                psO.tile([65, 512], f32, tag="po", name=f"po{hp}_{qb}_{i}")
                                for i in range(2)
                            ]
                            nkc = (qb + 1) * (QB // KC)
                            stage = []  # (ps, lh, kcp) pending exp+PV
                            for kcp in range(nkc // 2 + 1):
                                if kcp < nkc // 2:
                                    # filler BEFORE the S pairs; the four S
                                    # matmuls stay contiguous so the lh0/lh1
                                    # halves (row groups 0-1 vs 2-3, dh=64)
                                    # overlap in the PE array
                                    emit_filler(4)
                                    for lh in range(2):
                                        b0 = 64 * lh
                                        ps = psS.tile([128, 1024], f32)
                                        for j in range(2):
                                            kc = 2 * kcp + j
                                            o = kc * 128 - qb * 512
                                            lo = max(o, 0)
                                            pe_mask = o >= 0 and qb == 0
                                            if pe_mask:
                                                # qb0: seed -1e9 above the
                                                # diagonal on PE (a DVE mask
                                                # here would queue behind the
                                                # previous qb's epilogue)
                                                nc.tensor.matmul(
                                                    ps[:, j * 512 + lo : j * 512 + lo + 128],
                                                    ident_sb[:],
                                                    ntriu_sb[:],
                                                    start=True,
                                                    stop=False,
                                                )
                                            nc.tensor.matmul(
                                                ps[:, j * 512 + lo : (j + 1) * 512],
                                                qkT_sb[
                                                    b0 : b0 + 64,
                                                    4 + hp,
                                                    kc * 128 : (kc + 1) * 128,
                                                ],
                                                qkT_sb[
                                                    b0 : b0 + 64,
                                                    hp,
                                                    qb * 512 + lo : (qb + 1) * 512,
                                                ],
                                                start=(not pe_mask),
                                                stop=True,
                                            )
                                        stage.append((ps, lh, kcp))
                                if kcp > 0 and stage:
                                    ready, stage = stage[:2], stage[2:]
                                    for ps, lh, pkcp in ready:
                                        pt = pexp.tile([128, 1024], bf16)
                                        nc.scalar.activation(pt[:], ps[:], Exp)
                                        for j in range(2):
                                            o = (2 * pkcp + j) * 128 - qb * 512
                                            if o >= 0 and qb > 0:
                                                # diag mask on DVE (cheap; no
                                                # epilogue collision off qb0)
                                                nc.vector.tensor_mul(
                                                    pt[:, j * 512 + o : j * 512 + o + 128],
                                                    pt[:, j * 512 + o : j * 512 + o + 128],
                                                    tril_sb[:],
                                                )
                                        for j in range(2):
                                            kc = 2 * pkcp + j
                                            o = kc * 128 - qb * 512
                                            lo = max(o, 0)
                                            nc.tensor.matmul(
                                                po[lh][:, lo:512],
                                                V_sb[
                                                    :,
                                                    kc,
                                                    (2 * hp + lh) * 65 : (2 * hp + lh) * 65
                                                    + 65,
                                                ],
                                                pt[:, j * 512 + lo : (j + 1) * 512],
                                                start=(kc == 0),
                                                stop=(kc == nkc - 1),
                                            )

                            # per-(hp,qb) epilogue: reciprocal of rowsums
                            # straight from PSUM, evict unnormalized O^T,
                            # broadcast 1/rowsum, normalize in place (GPSIMD)
                            q_sl = slice(qb * 512, (qb + 1) * 512)
                            idx = hp * nqb + qb
                            rs = rsbp.tile([65, 512], f32, tag="rs")
                            for lh in range(2):
                                nc.vector.tensor_copy(
                                    rs[64 * lh : 64 * lh + 1, :], po[lh][64:65, :]
                                )
                            # broadcast RAW rowsums right away (DMA latency
                            # hides under the evict copies below), then a fast
                            # approx reciprocal on the 128-lane tile
                            rsb = rsbp.tile([128, 512], f32, tag="rsb")
                            for lh in range(2):
                                row = rs[64 * lh : 64 * lh + 1, :]
                                src = bass.AP(
                                    tensor=row.tensor,
                                    offset=row.offset,
                                    ap=[list(row.ap[0]), [0, 64], [1, 512]],
                                )
                                eng = (nc.sync, nc.gpsimd)[idx % 2]
                                eng.dma_start(rsb[64 * lh : 64 * lh + 64, :], src)
                            for lh in range(2):
                                nc.vector.tensor_copy(
                                    OT_sb[64 * lh : 64 * lh + 64, hp, q_sl],
                                    po[lh][0:64, :],
                                )
                            nc.vector.reciprocal_approx_fast(rsb[:, :], rsb[:, :])
                            nc.gpsimd.tensor_mul(
                                OT_sb[:, hp, q_sl],
                                OT_sb[:, hp, q_sl],
                                rsb[:, :],
                            )
                            if hp == 3:
                                # this qb's OT columns are now final for all
                                # head pairs: unlock its output-proj blocks
                                for tt in range(qb * 4, qb * 4 + 4):
                                    fst["yq"].append((tt, 0))
                                    fst["yq"].append((tt, 1))

                    # drain remaining output-projection work
                    fst["drain"] = True
                    while fst["yi"] < len(fst["yq"]) or fst["blk"] < len(fill_blocks):
                        emit_filler(4, junk_ok=False)

    nc.compile()
    return nc


def get_nc(T=2048, mm_dt="bf16", dbg=False):
    key = (T, mm_dt, dbg)
    if key not in _cache:
        _cache[key] = _build(T, mm_dt, dbg)
    return _cache[key]


def make_in_maps(x, qkv_w, qkv_b, proj_w, proj_b):
    B, T, _ = x.shape
    f = np.float32
    bf = ml_dtypes.bfloat16
    # S^T blocks are [key, query]: keep k <= q  ->  upper triangle
    tril = np.triu(np.ones((128, 128), f)).astype(bf)
    ident = np.eye(128, dtype=f).astype(bf)
    # -1e9 on the masked (k > q) strict-lower part of the S^T block
    ntriu = (-1e9 * (1.0 - np.triu(np.ones((128, 128), f)))).astype(bf)
    in_maps = []
    for i in range(B * 2):
        b, g = i // 2, i % 2
        sl = slice(g * 512, (g + 1) * 512)
        wq = qkv_w[0 * C : 1 * C][sl] * (1.0 / 8.0)
        wk = qkv_w[1 * C : 2 * C][sl]
        wv = qkv_w[2 * C : 3 * C][sl]
        in_maps.append(
            {
                "xT": np.ascontiguousarray(x[b].T).astype(bf),
                "wqkT": np.ascontiguousarray(
                    np.stack(
                        [
                            np.concatenate([wq, wk], 0)
                            .T[:, rt * 128 : (rt + 1) * 128]
                            .reshape(C // 128, 128, 128)
                            .transpose(1, 0, 2)
                            for rt in range(C // 128)
                        ]
                    )
                ).astype(bf),
                "wvT": np.ascontiguousarray(wv.T).astype(bf),
                "woT": np.ascontiguousarray(proj_w[:, sl].T).astype(bf),
                "qkb": np.concatenate(
                    [qkv_b[0 * C : 1 * C][sl] * (1.0 / 8.0), qkv_b[1 * C : 2 * C][sl]]
                ).astype(f),
                "vb": qkv_b[2 * C : 3 * C][sl].astype(f),
                "tril": tril,
                "ident": ident,
                "ntriu": ntriu,
            }
        )
    return in_maps


def kernel(x, qkv_w, qkv_b, proj_w, proj_b, mm_dt="bf16", trace=False, tmpdir=None):
    from concourse.bass_utils import run_bass_kernel_spmd

    x = np.asarray(x, np.float32)
    qkv_w = np.asarray(qkv_w, np.float32)
    qkv_b = np.asarray(qkv_b, np.float32)
    proj_w = np.asarray(proj_w, np.float32)
    proj_b = np.asarray(proj_b, np.float32)

    B, T, _ = x.shape
    nc = get_nc(T, mm_dt)
    in_maps = make_in_maps(x, qkv_w, qkv_b, proj_w, proj_b)
    res = run_bass_kernel_spmd(
        nc, in_maps, list(range(len(in_maps))), trace=trace, tmpdir=tmpdir
    )
    out = np.empty((B, T, C), np.float32)
    for b in range(B):
        out[b] = res.results[2 * b]["y"] + res.results[2 * b + 1]["y"] + proj_b
    kernel.last_result = res
    return out
